# revision 54
# baseline (speedup 1.0000x reference)
"""MASS variational distribution head: MOG class log-likelihood + log_softmax.

Takes FULL inputs, returns FULL output [B, C]. Class-sharded across 8
NeuronCores (13 padded classes per core), single NEFF, per-block
AllReduces of the class-softmax denominator before the final log_softmax.

Math per (class c, component k), all on device:
  A = L^{-1} via truncated Neumann (I+X)(I+X^2), X = I - L (unit diag)
  M = A^T A, v = M mu, s = mu^T v
  comp(x) = -0.5 x^T M x + v.x - 0.5 s + cconst   (cconst host-folded:
            SHIFT - 0.5 D log2pi - logdet + logmix)
  class_lp = logsumexp_k comp ; out = log_softmax_c class_lp

comp is evaluated as one feature matmul over banded-triangle features:
chunk t covers rows (i1, j) with i = 2t+i1; only j >= 2t contributes
(W rows j < 2t are scaled to 0; j in {2t, 2t+1} get -0.5, j >= 2t+2 get
-1.0 = mirror-folded). The matmul contracts partitions [2t:128] only,
so F rows [0:2t) are never built nor read. X = I - L is folded host-side
(no on-device subtracts). SHIFT makes exp() safe without max-subtraction.

ck layouts: natural ck paired 2-per-128 block; "ckq" order =
8q + 4h + p used for M rows / W cols / psum rows (affine DMA scatter).
"""
import functools
import numpy as np

B, D, C, K = 2048, 64, 100, 8
NCORES = 8
CP = 104                 # padded class count (8 * 13)
CC = CP // NCORES        # classes per core = 13
CKC = CC * K             # ck per core = 104
NPAIR = CKC // 2         # 52
NQ = NPAIR // 4          # 13 four-pair batches
NT = D * D // 128        # 32 quad feature chunks
NB = B // 512            # 4 psum column blocks
SHIFT = 100.0
LOG2PI = 1.8378770664093453
PAD_MU = 1.0e3
LN2 = 0.6931471805599453


@functools.lru_cache(maxsize=2)
def _build_nc():
    import concourse.bacc as bacc
    import concourse.mybir as mybir
    import concourse.tile as tile

    dt = mybir.dt
    AF = mybir.ActivationFunctionType
    nc = bacc.Bacc("TRN2", target_bir_lowering=False, debug=False,
                   num_devices=NCORES)

    Xp = nc.dram_tensor("Xp", [128, NPAIR * 128], dt.bfloat16, kind="ExternalInput")
    XpT = nc.dram_tensor("XpT", [128, NPAIR * 128], dt.bfloat16, kind="ExternalInput")
    xt = nc.dram_tensor("xt", [D, B], dt.bfloat16, kind="ExternalInput")
    mu_st = nc.dram_tensor("mu_st", [128, CKC], dt.float32, kind="ExternalInput")
    mu_stb = nc.dram_tensor("mu_stb", [128, CKC], dt.bfloat16, kind="ExternalInput")
    cconst = nc.dram_tensor("cconst", [1, CKC], dt.float32, kind="ExternalInput")
    eye4b = nc.dram_tensor("eye4b", [128, 512], dt.bfloat16, kind="ExternalInput")
    eye1b = nc.dram_tensor("eye1b", [128, 128], dt.bfloat16, kind="ExternalInput")
    oneskt = nc.dram_tensor("oneskt", [CKC, CC], dt.bfloat16, kind="ExternalInput")
    scol = nc.dram_tensor("scol", [128, NT], dt.float32, kind="ExternalInput")
    out = nc.dram_tensor("out", [CC, B], dt.float32, kind="ExternalOutput")

    with tile.TileContext(nc) as tc:
        with (
            tc.tile_pool(name="dram", bufs=1, space="DRAM") as dpool,
            tc.tile_pool(name="consts", bufs=1) as cpool,
            tc.tile_pool(name="chain", bufs=4) as chp,
            tc.tile_pool(name="msb", bufs=1) as mpool,
            tc.tile_pool(name="wt", bufs=1) as wpool,
            tc.tile_pool(name="fb", bufs=1) as fpool,
            tc.tile_pool(name="ep", bufs=1) as epool,
            tc.tile_pool(name="ps", bufs=1, space="PSUM") as psp,
        ):
            # ---------------- constants + inputs ----------------
            eye4b_s = cpool.tile([128, 512], dt.bfloat16)
            nc.sync.dma_start(eye4b_s[:], eye4b[:])

            PIECES = ((0, 512), (512, 2048), (2048, 3584), (3584, 6656))
            xp_ts, xpt_ts = [], []

            def load_piece(i):
                c0, c1 = PIECES[i]
                xp_i = cpool.tile([128, c1 - c0], dt.bfloat16,
                                  name=f"xp{i}")
                nc.sync.dma_start(xp_i[:], Xp[:, c0:c1])
                xp_ts.append(xp_i)
                xpt_i = cpool.tile([128, c1 - c0], dt.bfloat16,
                                   name=f"xpt{i}")
                nc.sync.dma_start(xpt_i[:], XpT[:, c0:c1])
                xpt_ts.append(xpt_i)

            load_piece(0)

            # xr = [x; x] stacked
            xr = fpool.tile([128, B], dt.bfloat16, tag="xr")
            nc.sync.dma_start(xr[0:D, :], xt[:])
            nc.sync.dma_start(xr[D:2 * D, :], xt[:])

            mu_stb_s = cpool.tile([128, CKC], dt.bfloat16)
            nc.sync.dma_start(mu_stb_s[:], mu_stb[:])
            load_piece(1)
            eye1b_s = cpool.tile([128, 128], dt.bfloat16)
            nc.sync.dma_start(eye1b_s[:], eye1b[:])
            oneskt_s = cpool.tile([CKC, CC], dt.bfloat16)
            nc.sync.dma_start(oneskt_s[:], oneskt[:])
            mu_st_s = cpool.tile([128, CKC], dt.float32)
            nc.sync.dma_start(mu_st_s[:], mu_st[:])
            cconst_s = cpool.tile([1, CKC], dt.float32)
            nc.sync.dma_start(cconst_s[:], cconst[:])
            scol_s = cpool.tile([128, NT], dt.float32)
            nc.sync.dma_start(scol_s[:], scol[:])
            load_piece(2)
            load_piece(3)

            def xp_view(tiles, q):
                for (c0, c1), tl in zip(PIECES, tiles):
                    if c0 <= 512 * q < c1:
                        return tl[:, 512 * q - c0:512 * q - c0 + 512]
                raise AssertionError

            # onesmask[:, 4b+j] = 1 iff j == b; lets the 4 denominator
            # matmuls accumulate into one [4, 512] psum tile
            onesmask = cpool.tile([CKC, 4 * NB], dt.bfloat16)
            nc.vector.memset(onesmask[:], 0.0)
            for b in range(NB):
                nc.vector.memset(onesmask[:, 5 * b:5 * b + 1], 1.0)
            ones128f = cpool.tile([128, 1], dt.float32)
            nc.vector.memset(ones128f[:], 1.0)
            ones2_s = cpool.tile([2, B], dt.bfloat16)
            nc.vector.memset(ones2_s[:], 1.0)

            # dummy collective issued before anything else reaches the Pool
            # queue: warms the CC mesh and absorbs cross-core launch skew
            dmy_sb = epool.tile([1, 8], dt.float32)
            nc.vector.memset(dmy_sb[:], 0.0)
            dmy_in = dpool.tile([1, 8], dt.float32, name="dmyinb")
            nc.sync.dma_start(dmy_in[:], dmy_sb[:])
            dmy_out = dpool.tile([1, 8], dt.float32, addr_space="Shared",
                                 name="dmyoutb")
            nc.gpsimd.collective_compute(
                "AllReduce", mybir.AluOpType.add,
                replica_groups=[list(range(NCORES))],
                ins=[dmy_in[:]], outs=[dmy_out[:]])

            # ---------------- features (banded, JIT) ----------------
            # feature chunk t rows (i1, j): i = 2t + i1; valid j >= 2t.
            # Rows [0:2t) never built nor contracted; middle garbage rows
            # [64:64+2t) (j' < 2t) are W-zeroed via scol.
            fts = [None] * NT
            xbs = [None] * NT
            # greedy engine balance in us-units: DVE ~1.63us / full tile,
            # Pool ~4.1us; DVE starts with the chain-add fixed load
            eng_load = {"v": 20.0, "g": 2.0}
            RATE = {"v": 2.17, "g": 4.24}

            def emit_bcast(t):
                # broadcast only the valid band rows [2t:D) / [D+2t:128);
                # everything else is W-zeroed and holds finite stale data
                # from the pool buffer's previous tile (DMA has no
                # partition-quadrant restriction, unlike compute engines).
                # The first 8 tiles broadcast in full: they initialize the
                # rotating buffers, so later tiles' dead rows are never
                # virgin SBUF (which could hold NaN bit patterns).
                r0 = 2 * t if t >= 8 else 0
                xb_t = fpool.tile([128, B], dt.bfloat16, tag="xb_t",
                                  bufs=8, name=f"xb_t{t}")
                nc.sync.dma_start(
                    xb_t[r0:D, :],
                    xt[2 * t:2 * t + 1, :].broadcast_to([D - r0, B]))
                nc.sync.dma_start(
                    xb_t[D + r0:128, :],
                    xt[2 * t + 1:2 * t + 2, :].broadcast_to([D - r0, B]))
                xbs[t] = xb_t

            NCHAIN_MUL = 5

            def emit_mul(t):
                # NOTE: DVE/Pool op time scales with the FREE dim only;
                # partition sub-ranges save nothing, so one full multiply
                f_t = fpool.tile([128, B], dt.bfloat16, tag="f_t",
                                 bufs=16, name=f"f_t{t}")
                cv = eng_load["v"] + RATE["v"]
                cg = eng_load["g"] + RATE["g"]
                # tiles emitted during the chain go to Pool unconditionally:
                # a DVE mul stalled on its broadcast would block chain adds
                if t < NCHAIN_MUL or cg < cv:
                    eng, key = nc.gpsimd, "g"
                else:
                    eng, key = nc.vector, "v"
                eng_load[key] += RATE[key]
                eng.tensor_mul(f_t[:], xbs[t][:], xr[:])
                fts[t] = f_t

            # ---------------- phase A: chain -> M, v ----------------
            Mdram = dpool.tile([128, NT * 128], dt.bfloat16, name="Mdram")
            Msb = mpool.tile([128, NT * 128], dt.bfloat16)   # [ckq, 64i+j]
            v2_ps = psp.tile([128, CKC], dt.float32, tag="aux", bufs=1)
            rb_done = 0
            ix2s = [None] * NQ
            abs_ = [None] * NQ
            x2ps = [None] * NQ
            aps = [None] * NQ
            bc_cursor = [0]
            mul_cursor = [0]

            def emit_feats(n):
                # broadcast DMAs run strictly under the xb buf count so a
                # pool-rotation wait can never head-block the SP queue
                for _ in range(n):
                    if mul_cursor[0] >= NT:
                        break
                    while (bc_cursor[0] < NT
                           and bc_cursor[0] < mul_cursor[0] + 7):
                        emit_bcast(bc_cursor[0])
                        bc_cursor[0] += 1
                    emit_mul(mul_cursor[0])
                    mul_cursor[0] += 1

            def stage_x2(q):
                x2_ps = psp.tile([128, 512], dt.float32, tag="big", bufs=6)
                for p in range(4):
                    sl = slice(128 * p, 128 * p + 128)
                    nc.tensor.matmul(x2_ps[:, sl], xp_view(xpt_ts, q)[:, sl],
                                     xp_view(xp_ts, q)[:, sl],
                                     start=True, stop=True)
                x2ps[q] = x2_ps

            def stage_ix2(q):
                ix2_q = chp.tile([128, 512], dt.bfloat16, tag="ix2")
                nc.vector.tensor_add(ix2_q[:], x2ps[q][:], eye4b_s[:])
                ix2s[q] = ix2_q

            def stage_a(q):
                a_ps = psp.tile([128, 512], dt.float32, tag="big", bufs=6)
                for p in range(4):
                    sl = slice(128 * p, 128 * p + 128)
                    nc.tensor.matmul(a_ps[:, sl], xp_view(xpt_ts, q)[:, sl],
                                     ix2s[q][:, sl], start=True, stop=True)
                aps[q] = a_ps

            def stage_ab(q):
                ab_q = chp.tile([128, 512], dt.bfloat16, tag="ab")
                nc.vector.tensor_add(ab_q[:], aps[q][:], ix2s[q][:])
                abs_[q] = ab_q

            mbs = [None] * NQ

            def stage_m(q):
                ab_q = abs_[q]
                m_ps = psp.tile([128, 512], dt.float32, tag="big", bufs=6)
                for p in range(4):
                    sl = slice(128 * p, 128 * p + 128)
                    nc.tensor.matmul(m_ps[:, sl], ab_q[:, sl], ab_q[:, sl],
                                     start=True, stop=True)
                mb_q = chp.tile([128, 512], dt.bfloat16, tag="mb", bufs=5)
                nc.scalar.activation(mb_q[:], m_ps[:], AF.Copy)
                mbs[q] = mb_q

                # scatter M diag-blocks -> Mdram rows 8q+4h+p (ckq order),
                # split per (h, p-half); issued on scalar right after the
                # mb copy (same engine, no cross-engine wait)
                for h in range(2):
                    src = mb_q[64 * h:64 * h + 64, :].rearrange(
                        "i (p c) -> i p c", c=128)[:, :, 64 * h:64 * h + 64]
                    for ph in range(2):
                        dst = Mdram[8 * q + 4 * h + 2 * ph:
                                    8 * q + 4 * h + 2 * ph + 2, :].rearrange(
                            "p (i j) -> i p j", j=64)
                        nc.scalar.dma_start(dst, src[:, 2 * ph:2 * ph + 2, :])

            def stage_v2(q):
                # v pair-matmuls, deferred one pipeline step so the PE
                # never waits on the scalar-engine mb copy of the SAME q
                mb_q = mbs[q]
                for p in range(4):
                    pr = 4 * q + p
                    nc.tensor.matmul(v2_ps[:, 2 * pr:2 * pr + 2],
                                     mb_q[:, 128 * p:128 * p + 128],
                                     mu_stb_s[:, 2 * pr:2 * pr + 2],
                                     start=True, stop=True)

            # software-pipelined chain (3-deep): q's are independent
            nonlocal_rb = [0]

            def maybe_readback(q):
                if q in (2, 5, 8, 11, 12):
                    r0, r1 = 8 * nonlocal_rb[0], 8 * q + 8
                    nc.scalar.dma_start(Msb[r0:r1, :], Mdram[r0:r1, :])
                    nonlocal_rb[0] = q + 1

            # only the first few (Pool) muls run during the chain: their
            # broadcasts are issued upfront and drain ahead of the scatter
            # descriptors; the big broadcast flood starts post-chain
            for t in range(NCHAIN_MUL):
                emit_bcast(t)
                bc_cursor[0] += 1

            for step in range(NQ + 3):
                if step < NQ:
                    stage_x2(step)
                    stage_ix2(step)
                if 1 <= step <= NQ:
                    stage_a(step - 1)
                    stage_ab(step - 1)
                if 2 <= step <= NQ + 1:
                    stage_m(step - 2)
                if 3 <= step:
                    stage_v2(step - 3)
                    maybe_readback(step - 3)
                if step % 3 == 2 and mul_cursor[0] < NCHAIN_MUL:
                    emit_mul(mul_cursor[0])
                    mul_cursor[0] += 1

            emit_feats(NT)  # any remaining

            # phase B: v2zb, s, const rows — emitted LATE (inside the main
            # matmul chunk loop) so its DVE/PE ops queue behind the feature
            # muls without blocking the W-transpose -> main-mm PE stream
            def ckq_view(row_ap, h):
                return row_ap.rearrange("r (q hh p) -> r q hh p",
                                        hh=2, p=4)[:, :, h, :]

            def nat_view(row_ap, h):
                return row_ap[:, h::2].rearrange("r (q p) -> r q p", p=4)

            phase_b_out = {}

            def emit_phase_b():
                v2zb = wpool.tile([128, CKC], dt.bfloat16, tag="v2zb")
                for h in range(2):
                    nc.vector.tensor_copy(ckq_view(v2zb[:], h),
                                          nat_view(v2_ps[:], h))
                mv2 = epool.tile([128, CKC], dt.float32)
                nc.vector.tensor_mul(mv2[:], v2_ps[:], mu_st_s[:])
                s_ps = psp.tile([1, CKC], dt.float32, tag="aux", bufs=1)
                nc.tensor.matmul(s_ps[:], ones128f[:], mv2[:],
                                 start=True, stop=True)
                crow3 = epool.tile([1, CKC], dt.float32)
                nc.vector.scalar_tensor_tensor(
                    crow3[:], s_ps[:], -0.5, cconst_s[:],
                    op0=mybir.AluOpType.mult, op1=mybir.AluOpType.add)
                c2r = wpool.tile([2, CKC], dt.bfloat16, tag="c2r")
                crem = epool.tile([1, CKC], dt.float32)
                for h in range(2):
                    nc.vector.tensor_copy(ckq_view(c2r[0:1, :], h),
                                          nat_view(crow3[:], h))
                    nc.vector.tensor_sub(ckq_view(crem[:], h),
                                         nat_view(crow3[:], h),
                                         ckq_view(c2r[0:1, :], h))
                cremb = epool.tile([1, CKC], dt.bfloat16)
                nc.vector.tensor_copy(cremb[:], crem[:])
                nc.scalar.dma_start(c2r[1:2, :], cremb[:])
                phase_b_out["v2zb"] = v2zb
                phase_b_out["c2r"] = c2r

            # ---------------- W tiles: PE transpose + banded scale ----------------
            wts = []
            for t2 in range(0, NT, 2):
                tp_ps = psp.tile([128, 256], dt.bfloat16, tag="ks", bufs=1)
                for j in range(2):
                    nc.tensor.transpose(
                        tp_ps[:, 128 * j:128 * j + 128],
                        Msb[:, 128 * (t2 + j):128 * (t2 + j) + 128],
                        eye1b_s[:])
                wt2 = wpool.tile([128, 256], dt.bfloat16, tag=f"wt{t2}",
                                 name=f"wt{t2}")
                for j in range(2):
                    nc.scalar.mul(wt2[:, 128 * j:128 * j + 128],
                                  tp_ps[:, 128 * j:128 * j + 128],
                                  scol_s[:, t2 + j:t2 + j + 1])
                wts.append(wt2[:, 0:CKC])
                wts.append(wt2[:, 128:128 + CKC])

            # ---------------- phase C: main matmul (chunk-major) ----------------
            s_pss = [psp.tile([CKC, 512], dt.float32, tag="big", bufs=6,
                              name=f"spsum{b}") for b in range(NB)]
            # q-chunks first: chunk t's matmul frees f_t for the rotating
            # feature pool; xr/c last (v2zb/c2r land late on DVE)
            chunks = [("q", t) for t in range(NT)] + [("xr", -1), ("c", -1)]
            NCH = len(chunks)
            for ci, (kind, t) in enumerate(chunks):
                first, last = ci == 0, ci == NCH - 1
                if kind == "xr":
                    emit_phase_b()
                for b in range(NB):
                    bs = slice(512 * b, 512 * b + 512)
                    if kind == "q":
                        nc.tensor.matmul(s_pss[b][:], wts[t],
                                         fts[t][:, bs],
                                         start=first, stop=last)
                    elif kind == "xr":
                        nc.tensor.matmul(s_pss[b][:], phase_b_out["v2zb"][:],
                                         xr[:, bs],
                                         start=first, stop=last)
                    else:
                        nc.tensor.matmul(s_pss[b][:], phase_b_out["c2r"][:],
                                         ones2_s[:, bs],
                                         start=first, stop=last)

            # ---------------- phase D: per-block epilogue ----------------
            def safe_ln(out_ap, src_ap, pfx, veng):
                # out = ln(src) + 127*ln2, exact for any positive fp32.
                P, N = src_ap.shape[0], src_ap.shape[-1]
                xb_ = src_ap.bitcast(dt.int32)
                sh = epool.tile([P, N], dt.int32, tag="slsh", bufs=2,
                                name=f"{pfx}sh")
                nc.vector.tensor_scalar(
                    sh[:], xb_, 23, None,
                    op0=mybir.AluOpType.logical_shift_right)
                ef = epool.tile([P, N], dt.float32, tag="slef", bufs=2,
                                name=f"{pfx}ef")
                veng.tensor_copy(ef[:], sh[:])
                mi = epool.tile([P, N], dt.int32, tag="slmi", bufs=2,
                                name=f"{pfx}mi")
                nc.vector.tensor_scalar(
                    mi[:], xb_, 0x007FFFFF, 0x3F800000,
                    op0=mybir.AluOpType.bitwise_and,
                    op1=mybir.AluOpType.bitwise_or)
                lnm = epool.tile([P, N], dt.float32, tag="sllnm", bufs=2,
                                 name=f"{pfx}lnm")
                nc.scalar.activation(lnm[:], mi[:].bitcast(dt.float32), AF.Ln)
                nc.vector.scalar_tensor_tensor(
                    out_ap, ef[:], LN2, lnm[:],
                    op0=mybir.AluOpType.mult, op1=mybir.AluOpType.add)

            # exp all blocks, then the denominator path first (ACT+PE+CC
            # only) so the single AllReduce overlaps the numerator work
            E = epool.tile([CKC, B], dt.bfloat16)
            for b in range(NB):
                bs = slice(512 * b, 512 * b + 512)
                nc.scalar.activation(E[:, bs], s_pss[b][:], AF.Exp)

            dn4_ps = psp.tile([NB, 512], dt.float32, tag="ks", bufs=1,
                              name="dn4ps")
            for b in range(NB):
                bs = slice(512 * b, 512 * b + 512)
                nc.tensor.matmul(dn4_ps[:], onesmask[:, 4 * b:4 * b + 4],
                                 E[:, bs], start=(b == 0), stop=(b == NB - 1))
            crin4 = epool.tile([NB, 512], dt.float32)
            nc.scalar.copy(crin4[:], dn4_ps[:])
            crin_d = dpool.tile([NB, 512], dt.float32, name="crin")
            nc.sync.dma_start(crin_d[:], crin4[:])
            crout_d = dpool.tile([NB, 512], dt.float32,
                                 addr_space="Shared", name="crout")
            nc.gpsimd.collective_compute(
                "AllReduce", mybir.AluOpType.add,
                replica_groups=[list(range(NCORES))],
                ins=[crin_d[:]], outs=[crout_d[:]])

            # numerator lns overlap the AllReduce wait
            cl_sb = []
            for b in range(NB):
                bs = slice(512 * b, 512 * b + 512)
                ks_ps = psp.tile([CC, 512], dt.float32, tag="ks", bufs=1,
                                 name=f"ksps{b}")
                nc.tensor.matmul(ks_ps[:], oneskt_s[:], E[:, bs],
                                 start=True, stop=True)
                cl_b = epool.tile([CC, 512], dt.float32, tag=f"cl{b}",
                                  name=f"cl{b}")
                safe_ln(cl_b[:], ks_ps[:], f"s1{b}",
                        nc.vector if (b % 2 == 0) else nc.gpsimd)
                cl_sb.append(cl_b)

            # broadcast the raw AllReduced denominator row straight from
            # DRAM over the CC class partitions, ln it there, subtract
            for b in range(NB):
                bs = slice(512 * b, 512 * b + 512)
                ldb = epool.tile([CC, 512], dt.float32, tag="ldb", bufs=2,
                                 name=f"ldb{b}")
                nc.scalar.dma_start(
                    ldb[:], crout_d[b:b + 1, :].broadcast_to([CC, 512]))
                lnd = epool.tile([CC, 512], dt.float32, tag="lnd", bufs=2,
                                 name=f"lnd{b}")
                safe_ln(lnd[:], ldb[:], f"s2{b}",
                        nc.vector if (b % 2 == 0) else nc.gpsimd)
                lg_b = epool.tile([CC, 512], dt.float32, tag="lgb", bufs=2,
                                  name=f"lgb{b}")
                eng = nc.vector if (b % 2 == 0) else nc.gpsimd
                eng.tensor_sub(lg_b[:], cl_sb[b][:], lnd[:])
                nc.sync.dma_start(out[:, bs], lg_b[:])

    if not nc.is_finalized():
        nc.finalize()
    return nc


def _prep_inputs(representation, mixture_logits, loc, scale_tril):
    import ml_dtypes
    bf16 = ml_dtypes.bfloat16
    f32 = np.float32

    pad = CP - C
    mixp = np.concatenate([np.asarray(mixture_logits, f32),
                           np.zeros((pad, K), f32)], 0)
    locp = np.concatenate([np.asarray(loc, f32),
                           np.full((pad, K, D), PAD_MU, f32)], 0)
    eye = np.eye(D, dtype=f32)
    stp = np.concatenate([np.asarray(scale_tril, f32),
                          np.broadcast_to(eye, (pad, K, D, D)).copy()], 0)

    xtb = np.ascontiguousarray(np.asarray(representation, f32).T).astype(bf16)

    eye4 = np.zeros((128, 512), f32)
    for p in range(4):
        eye4[:, 128 * p:128 * p + 128] = np.eye(128, dtype=f32)
    eye4 = eye4.astype(bf16)
    eye1 = np.eye(128, dtype=f32).astype(bf16)

    # host-folded per-(c,k) constants: SHIFT - D/2 log2pi - logdet + logmix
    dg = np.diagonal(stp, axis1=2, axis2=3)                     # [CP, K, D]
    logdet = np.log(np.abs(dg.astype(np.float64))).sum(-1)      # [CP, K]
    mx = mixp.astype(np.float64)
    logmix = mx - np.log(np.exp(mx - mx.max(-1, keepdims=True)).sum(
        -1, keepdims=True)) - mx.max(-1, keepdims=True)
    ccf = (SHIFT - 0.5 * D * LOG2PI - logdet + logmix).astype(f32)  # [CP, K]

    # ckq permutation: ck = 8q + 2p + h -> ckq = 8q + 4h + p
    onesk = np.zeros((CKC, CC), f32)
    for ck in range(CKC):
        q_, rem = divmod(ck, 8)
        p_, h_ = divmod(rem, 2)
        onesk[8 * q_ + 4 * h_ + p_, ck // K] = 1.0
    onesk = onesk.astype(bf16)

    # banded W per-partition scale: chunk t row p = 64*i1 + j:
    #   j < 2t -> 0 ; j in {2t, 2t+1} -> -0.5 ; j >= 2t+2 -> -1.0
    scolf = np.zeros((128, NT), f32)
    for t in range(NT):
        for i1 in range(2):
            for j in range(D):
                if j < 2 * t:
                    v = 0.0
                elif j <= 2 * t + 1:
                    v = -0.5
                else:
                    v = -1.0
                scolf[64 * i1 + j, t] = v

    in_maps = []
    for r in range(NCORES):
        cls = slice(CC * r, CC * r + CC)
        Lck = stp[cls].reshape(CKC, D, D)
        muck = locp[cls].reshape(CKC, D)
        Xpq = np.zeros((NPAIR, 128, 128), f32)
        XpqT = np.zeros((NPAIR, 128, 128), f32)
        eyeD = np.eye(D, dtype=f32)
        for m in range(NPAIR):
            X0 = eyeD - Lck[2 * m]
            X1 = eyeD - Lck[2 * m + 1]
            Xpq[m, 0:D, 0:D] = X0
            Xpq[m, D:2 * D, D:2 * D] = X1
            XpqT[m, 0:D, 0:D] = X0.T
            XpqT[m, D:2 * D, D:2 * D] = X1.T
        Xp2 = np.ascontiguousarray(Xpq.transpose(1, 0, 2).reshape(128, -1))
        Xp2T = np.ascontiguousarray(XpqT.transpose(1, 0, 2).reshape(128, -1))
        must = np.zeros((128, CKC), f32)
        for ck in range(CKC):
            hh = ck % 2
            must[64 * hh:64 * hh + 64, ck] = muck[ck]
        in_maps.append({
            "Xp": Xp2.astype(bf16),
            "XpT": Xp2T.astype(bf16),
            "xt": xtb,
            "mu_st": must,
            "mu_stb": must.astype(bf16),
            "cconst": np.ascontiguousarray(
                ccf[cls].reshape(1, CKC)),
            "eye4b": eye4,
            "eye1b": eye1,
            "oneskt": onesk,
            "scol": scolf,
        })
    return in_maps


def _postprocess(results):
    rows = [results[r]["out"] for r in range(NCORES)]
    full = np.concatenate(rows, 0)[:C]
    return np.ascontiguousarray(full.T).astype(np.float32)


def kernel(representation, mixture_logits, loc, scale_tril):
    from concourse.bass_utils import run_bass_kernel_spmd
    nc = _build_nc()
    in_maps = _prep_inputs(representation, mixture_logits, loc, scale_tril)
    res = run_bass_kernel_spmd(nc, in_maps, core_ids=list(range(NCORES)))
    return _postprocess(res.results)
